# revision 16
# baseline (speedup 1.0000x reference)
"""Contrastive-loss kernel for 8 Trainium2 NeuronCores — v2.

Math (reference):
    sim = X @ X.T; pos = targets outer-equal
    loss = ( sum(pos & sim<1 -> 1-sim) + sum(~pos & sim>m -> sim) ) / n

v2 strategy (vs v1's full-matrix data-parallel):
  * Host sorts rows by class label -> positive pairs live in a narrow
    diagonal band (max class size <= 128 asserted).
  * Symmetry: each 128-column i-block b computes only the cyclic window
    j in [c0-4096, c0+128) (c0 = 128*b).  Unordered block pairs at cyclic
    distance 1..31 are computed once (weight 2), distance 32 twice
    (weight 1 each, "phase 0"), the diagonal block once (weight 1 via
    masks).  ~53% of the baseline matmul volume.
  * Work split: core r owns i-blocks {4r..4r+3} and {60-4r..63-4r}
    (stripes r and 15-r) -> every core: 8 blocks x 4224-wide windows,
    a fully uniform SPMD program.  Host pre-rotates X.T per stripe so
    all windows are compile-time slices.
  * Per 2048-wide PSUM tile: ACT does relu(s-m) with accum_out (row sums
    of u), DVE does (s > m) count with accum_out.  The label structure
    enters ONLY through the last 256 columns of each block's window
    ("band"): three tensor_tensor_reduce ops against host-built masks
    mf (f_neg correction) and mp (f_pos weights) fix up positive pairs
    and the diagonal exactly.
  * Final reduction on host from per-core [128, 72] accumulator columns.
"""

import numpy as np
import ml_dtypes

N = 8192
D = 512
C = 128
NCORES = 8
KT = 4            # k tiles (contraction 512 = 4 x 128)
NB = 8            # i-blocks per core
WIN = 4224        # per-block j window (128 phase0 + 2 x 2048 main)
XW = 4608         # rotated xt width per stripe (windows + stationaries)
MARGIN = 0.3
MMW = 512         # moving width per matmul (PSUM bank limit)

_BF16 = ml_dtypes.bfloat16

_COMPILED = None
LAST_RESULTS = None

# accumulator column maps
#   accA (ACT, f32): 0 = phase0 u (w1); per block k: 1+3k = T1u, 2+3k = T2u,
#       3+3k = T1 sign(s-m) sum (count = (S + 128*2048)/2 on host)
#   accD (DVE, f32): 0 = phase0 count (w1); per block k:
#       1+4k = T2c count, 2+4k = mf*u, 3+4k = mf*c, 4+4k = mp*a
NA = 32
ND = 40


def _build():
    import concourse.tile as tile
    from concourse import bacc, mybir

    nc = bacc.Bacc("TRN2", target_bir_lowering=False, debug=False,
                   num_devices=NCORES)
    bf16 = mybir.dt.bfloat16
    f32 = mybir.dt.float32
    relu = mybir.ActivationFunctionType.Relu
    sign = mybir.ActivationFunctionType.Sign
    alu = mybir.AluOpType

    xta_d = nc.dram_tensor("xta", [D, XW], bf16, kind="ExternalInput").ap()
    xtb_d = nc.dram_tensor("xtb", [D, XW], bf16, kind="ExternalInput").ap()
    mf_d = nc.dram_tensor("mf", [128, NB * 256], bf16, kind="ExternalInput").ap()
    mp_d = nc.dram_tensor("mp", [128, NB * 256], bf16, kind="ExternalInput").ap()
    out_d = nc.dram_tensor("out", [128, NA + ND], f32, kind="ExternalOutput").ap()

    with tile.TileContext(nc) as tc:
        with (
            tc.tile_pool(name="xt", bufs=1) as xt_pool,
            tc.tile_pool(name="msk", bufs=1) as msk_pool,
            tc.tile_pool(name="acc", bufs=1) as acc_pool,
            tc.tile_pool(name="u", bufs=3) as u_pool,
            tc.tile_pool(name="c", bufs=3) as c_pool,
            tc.tile_pool(name="band", bufs=2) as band_pool,
            tc.tile_pool(name="ps_main", bufs=2, space="PSUM") as ps_main,
        ):
            xta_sb = [xt_pool.tile([128, XW], bf16, name=f"xta{k}",
                                   tag=f"xta{k}") for k in range(KT)]
            xtb_sb = [xt_pool.tile([128, XW], bf16, name=f"xtb{k}",
                                   tag=f"xtb{k}") for k in range(KT)]

            def load(sb, dram, lo, hi):
                for kt in range(KT):
                    nc.sync.dma_start(sb[kt][:, lo:hi],
                                      dram[kt * 128:(kt + 1) * 128, lo:hi])

            # stationaries + phase0 columns first so PE can start ASAP
            load(xta_sb, xta_d, 4096, XW)
            load(xta_sb, xta_d, 0, 512)
            load(xtb_sb, xtb_d, 4096, XW)
            load(xtb_sb, xtb_d, 0, 512)

            # remaining moving columns, block-0-first order; masks are only
            # needed at block 0's T2 band ops so they go after T1's data
            load(xta_sb, xta_d, 512, 2176)
            mf_sb = msk_pool.tile([128, NB, 256], bf16)
            mp_sb = msk_pool.tile([128, NB, 256], bf16)
            nc.sync.dma_start(mf_sb[:], mf_d.rearrange("p (b w) -> p b w", b=NB))
            nc.sync.dma_start(mp_sb[:], mp_d.rearrange("p (b w) -> p b w", b=NB))
            load(xta_sb, xta_d, 2176, 4096)
            load(xtb_sb, xtb_d, 512, 2176)
            load(xtb_sb, xtb_d, 2176, 4096)

            accA = acc_pool.tile([128, NA], f32)
            accD = acc_pool.tile([128, ND], f32)
            bias_m = acc_pool.tile([128, 1], f32)
            nc.vector.memset(accA[:], 0.0)
            nc.vector.memset(accD[:], 0.0)
            nc.vector.memset(bias_m[:], -MARGIN)

            def src_of(kb):
                return (xta_sb if kb < 4 else xtb_sb), (kb % 4)

            # ---- phase 0: distance-32 blocks (weight 1), PE warmup ----
            ps0 = ps_main.tile([128, 1024], f32, name="ps0", tag="ps")
            for kb in range(NB):
                sb, kk = src_of(kb)
                st = 4096 + 128 * kk
                for kt in range(KT):
                    nc.tensor.matmul(
                        ps0[:, 128 * kb:128 * (kb + 1)],
                        lhsT=sb[kt][:, st:st + 128],
                        rhs=sb[kt][:, 128 * kk:128 * kk + 128],
                        start=(kt == 0), stop=(kt == KT - 1),
                    )
            u0 = u_pool.tile([128, 1024], bf16, tag="u")
            nc.scalar.activation(u0[:], ps0[:], relu,
                                 bias=bias_m[:], scale=1.0,
                                 accum_out=accA[:, 0:1])
            c0 = c_pool.tile([128, 1024], bf16, tag="c")
            nc.vector.tensor_scalar(c0[:], ps0[:], MARGIN, None,
                                    op0=alu.is_gt, op1=alu.add,
                                    accum_out=accD[:, 0:1])

            # ---- main: per block, two 2048-wide psum tiles ----
            for kb in range(NB):
                sb, kk = src_of(kb)
                st = 4096 + 128 * kk
                for half in range(2):
                    ps = ps_main.tile([128, 2048], f32, tag="ps")
                    w0 = 128 + 2048 * half + 128 * kk
                    for kt in range(KT):
                        for cs in range(2048 // MMW):
                            nc.tensor.matmul(
                                ps[:, MMW * cs:MMW * (cs + 1)],
                                lhsT=sb[kt][:, st:st + 128],
                                rhs=sb[kt][:, w0 + MMW * cs:w0 + MMW * (cs + 1)],
                                start=(kt == 0), stop=(kt == KT - 1),
                            )
                    ut = u_pool.tile([128, 2048], bf16, tag="u")
                    nc.scalar.activation(ut[:], ps[:], relu,
                                         bias=bias_m[:], scale=1.0,
                                         accum_out=accA[:, 1 + 3 * kb + half:
                                                        2 + 3 * kb + half])
                    ct = c_pool.tile([128, 2048], bf16, tag="c")
                    if half == 0:
                        # count via ACT Sign to keep DVE off the critical
                        # path: host recovers count = (S + 128*2048)/2
                        nc.scalar.activation(ct[:], ps[:], sign,
                                             bias=bias_m[:], scale=1.0,
                                             accum_out=accA[:, 3 + 3 * kb:
                                                            4 + 3 * kb])
                    else:
                        nc.vector.tensor_scalar(ct[:], ps[:], MARGIN, None,
                                                op0=alu.is_gt, op1=alu.add,
                                                accum_out=accD[:, 1 + 4 * kb:
                                                               2 + 4 * kb])
                    if half == 1:
                        at = band_pool.tile([128, 256], bf16, tag="a")
                        nc.scalar.activation(at[:], ps[:, 1792:2048], relu,
                                             bias=1.0, scale=-1.0)
                        # mf uses MARGIN-scaled c: fold m into the mask on
                        # the c term by accumulating count*mf separately
                        s1 = band_pool.tile([128, 256], bf16, tag="s1")
                        nc.vector.tensor_tensor(s1[:], ut[:, 1792:2048],
                                                mf_sb[:, kb, :], op=alu.mult)
                        nc.vector.tensor_reduce(
                            accD[:, 2 + 4 * kb:3 + 4 * kb], s1[:],
                            axis=mybir.AxisListType.X, op=alu.add)
                        s2 = band_pool.tile([128, 256], bf16, tag="s2")
                        nc.vector.tensor_tensor(s2[:], ct[:, 1792:2048],
                                                mf_sb[:, kb, :], op=alu.mult)
                        nc.vector.tensor_reduce(
                            accD[:, 3 + 4 * kb:4 + 4 * kb], s2[:],
                            axis=mybir.AxisListType.X, op=alu.add)
                        s3 = band_pool.tile([128, 256], bf16, tag="s3")
                        nc.vector.tensor_tensor(s3[:], at[:],
                                                mp_sb[:, kb, :], op=alu.mult)
                        nc.vector.tensor_reduce(
                            accD[:, 4 + 4 * kb:5 + 4 * kb], s3[:],
                            axis=mybir.AxisListType.X, op=alu.add)

            nc.sync.dma_start(out_d[:, 0:NA], accA[:])
            nc.sync.dma_start(out_d[:, NA:NA + ND], accD[:])

    nc.compile()
    return nc


def kernel(inputs, targets):
    global _COMPILED, LAST_RESULTS
    from concourse.bass_utils import run_bass_kernel_spmd

    X = np.asarray(inputs, dtype=np.float32)
    t = np.asarray(targets).astype(np.int64)
    assert X.shape == (N, D) and t.shape == (N,)

    order = np.argsort(t, kind="stable")
    ts = t[order]
    assert np.bincount(ts, minlength=C).max() <= 128, "class straddle > 128"
    XT = np.ascontiguousarray(X[order].astype(_BF16).T)      # [512, 8192]

    if _COMPILED is None:
        _COMPILED = _build()
    nc = _COMPILED

    in_maps = []
    for r in range(NCORES):
        m = {}
        for name, sblk in (("xta", 4 * r), ("xtb", 60 - 4 * r)):
            base = 128 * sblk              # global col of the stripe start
            rot = (base - 4096) % N        # rotated col 0 = this global col
            idx = (rot + np.arange(XW)) % N
            m[name] = np.ascontiguousarray(XT[:, idx])
        mf = np.zeros((128, NB, 256), dtype=_BF16)
        mp = np.zeros((128, NB, 256), dtype=_BF16)
        blocks = [4 * r + k for k in range(4)] + \
                 [60 - 4 * r + k for k in range(4)]
        for kb, b in enumerate(blocks):
            c0 = 128 * b
            jb = (np.arange(c0 - 128, c0 + 128)) % N
            same = ts[c0:c0 + 128][:, None] == ts[jb][None, :]
            w = np.where(np.arange(256) < 128, 2.0, 1.0)[None, :]
            mf[:, kb, :] = np.where(same, -2.0, w - 2.0).astype(_BF16)
            mp[:, kb, :] = np.where(same, w, 0.0).astype(_BF16)
        m["mf"] = np.ascontiguousarray(mf.reshape(128, NB * 256))
        m["mp"] = np.ascontiguousarray(mp.reshape(128, NB * 256))
        in_maps.append(m)

    res = run_bass_kernel_spmd(nc, in_maps, list(range(NCORES)))
    LAST_RESULTS = res

    m64 = np.float64(np.float32(MARGIN))
    total = np.float64(0.0)
    for r in range(NCORES):
        o = res.results[r]["out"].astype(np.float64).sum(axis=0)
        A, Dv = o[:NA], o[NA:]
        total += A[0] + m64 * Dv[0]                      # phase0, w1
        for kb in range(NB):
            total += 2.0 * (A[1 + 3 * kb] + A[2 + 3 * kb])
            cnt1 = (A[3 + 3 * kb] + 128.0 * 2048.0) / 2.0  # sign -> count
            total += 2.0 * m64 * (cnt1 + Dv[1 + 4 * kb])
            total += Dv[2 + 4 * kb] + m64 * Dv[3 + 4 * kb] + Dv[4 + 4 * kb]
    return np.asarray(total / N, dtype=np.float32)
